# revision 10
# baseline (speedup 1.0000x reference)
"""GQA with RoPE + sliding-window causal attention on 8 TRN2 NeuronCores.

Sharding: batch (2) x KV-groups (4) -> 8 cores, pure SPMD (no collectives).
Each core computes q/k/v projections for its (batch, group), RoPE, windowed
attention (window=512), and a partial output projection against its group's
WO columns. Host sums the 4 group partials per batch element.

v2 layout/perf notes:
  * All 2-byte tensors are fp16 (same PE/DVE rate as bf16, 8x the mantissa):
    x tiles, W_qkv, W_o, qk_sb, v_sb, masks, probs, attn.
  * Weights pre-permuted on host so each head's 64 dims are deinterleaved
    ([even | odd]) -> RoPE is two contiguous 32-wide halves per DVE op.
  * cos/sin stored deduplicated [128, NT, 1, 32] f32 and broadcast across
    the 5 heads with a stride-0 AP dim.
  * Both phases are software-pipelined so the PE stream stays dense:
    phase 1 issues proj(i) then transpose/drain(i-1); phase 2 issues
    scores/exp/mask(i) then AV..WO(i-1).
  * WO partials drain PSUM->SBUF as fp16 into one [128,1024] buffer
    (Pool engine), one output DMA per row tile; host sums fp16 partials.
"""

import sys

sys.path.insert(0, "/opt/trn_rl_repo")

import numpy as np
from contextlib import ExitStack

D_MODEL = 1024
GROUP_SIZE = 4
NUM_GROUPS = 4
D_K = 64
THETA = 10000.0
WINDOW = 512
T = 2048
B = 2
NT = T // 128  # 16 row tiles
HALF = D_K // 2

_PROGRAM = None


def _build_program():
    from concourse import bacc, tile
    import concourse.mybir as mybir

    f32 = mybir.dt.float32
    f16 = mybir.dt.float16
    Exp = mybir.ActivationFunctionType.Exp
    mult = mybir.AluOpType.mult
    subtract = mybir.AluOpType.subtract
    add = mybir.AluOpType.add

    nc = bacc.Bacc("TRN2", target_bir_lowering=False, debug=False, num_devices=8)

    xt_d = nc.dram_tensor("xt", [NT // 2, 128, 2, 8, 128], f16, kind="ExternalInput").ap()
    wq_d = nc.dram_tensor("wqkvT", [128, 8, 384], f16, kind="ExternalInput").ap()
    wo_d = nc.dram_tensor("woT", [128, 2, 1024], f16, kind="ExternalInput").ap()
    cos_d = nc.dram_tensor("cosb", [128, NT, 1, HALF], f32, kind="ExternalInput").ap()
    sin_d = nc.dram_tensor("sinb", [128, NT, 1, HALF], f32, kind="ExternalInput").ap()
    md_d = nc.dram_tensor("maskd", [128, 256], f16, kind="ExternalInput").ap()
    mo_d = nc.dram_tensor("masko", [128, 256], f16, kind="ExternalInput").ap()
    id16_d = nc.dram_tensor("ident16", [128, 128], f16, kind="ExternalInput").ap()
    out_d = nc.dram_tensor("out", [T, D_MODEL], f16, kind="ExternalOutput").ap()

    with tile.TileContext(nc) as tc:
        with ExitStack() as ctx:
            persist = ctx.enter_context(tc.tile_pool(name="persist", bufs=1))
            wq_sb = persist.tile([128, 8, 384], f16, tag="wq")
            wo_sb = persist.tile([128, 2, 1024], f16, tag="wo")
            cos_sb = persist.tile([128, NT, 1, HALF], f32, tag="cos")
            sin_sb = persist.tile([128, NT, 1, HALF], f32, tag="sin")
            md_sb = persist.tile([128, 256], f16, tag="md")
            mo_sb = persist.tile([128, 256], f16, tag="mo")
            id16_sb = persist.tile([128, 128], f16, tag="id16")
            qk_sb = persist.tile([64, 5, T], f16, tag="qk")  # dims-major q(4)+k
            v_sb = persist.tile([128, NT, 65], f16, tag="v")  # [v | 1] per key block

            # preloads needed by phase 1 first; wo/masks late (phase 2 use)
            nc.sync.dma_start(wq_sb[:], wq_d[:])
            nc.sync.dma_start(id16_sb[:], id16_d[:])
            nc.sync.dma_start(cos_sb[:], cos_d[:])
            nc.sync.dma_start(sin_sb[:], sin_d[:])

            # ---------------- phase 1: QKV projection + RoPE + transposes
            with ExitStack() as c1:
                xt_pool = c1.enter_context(tc.tile_pool(name="xtp", bufs=2))
                rot_pool = c1.enter_context(tc.tile_pool(name="rotp", bufs=2))
                tmp_pool = c1.enter_context(tc.tile_pool(name="tmpp", bufs=2))
                pp_pool = c1.enter_context(
                    tc.tile_pool(name="ppp", bufs=2, space="PSUM")
                )
                ptr_pool = c1.enter_context(
                    tc.tile_pool(name="ptrp", bufs=2, space="PSUM")
                )

                nc.vector.memset(v_sb[:, :, 64:65], 1.0)

                prev = None  # (rot, tt) awaiting PE transpose + drain

                def flush_prev():
                    nonlocal prev
                    if prev is None:
                        return
                    rot_p, tp = prev
                    pt = ptr_pool.tile([64, 5, 128], f16, tag="pt")
                    for hh in range(5):
                        nc.tensor.transpose(pt[:, hh, :], rot_p[:, hh, :], id16_sb[:])
                    nc.scalar.copy(qk_sb[:, :, tp * 128 : (tp + 1) * 128], pt[:])
                    prev = None

                xt = None
                for tt in range(NT):
                    if tt % 2 == 0:
                        xt = xt_pool.tile([128, 2, 8, 128], f16, tag="xt")
                        nc.sync.dma_start(xt[:], xt_d[tt // 2])
                        if tt == 0:
                            nc.sync.dma_start(wo_sb[:], wo_d[:])
                            nc.sync.dma_start(md_sb[:], md_d[:])
                            nc.sync.dma_start(mo_sb[:], mo_d[:])
                    pp = pp_pool.tile([128, 6, 64], f32, tag="pp")
                    for kt in range(8):
                        nc.tensor.matmul(
                            pp[:],
                            lhsT=xt[:, tt % 2, kt, :],
                            rhs=wq_sb[:, kt, :],
                            start=(kt == 0),
                            stop=(kt == 7),
                        )
                    # PE: transposes of previous tile (its RoPE is done)
                    flush_prev()
                    # DVE/Pool: RoPE of this tile
                    a = pp[:, 0:5, 0:HALF]
                    b = pp[:, 0:5, HALF:D_K]
                    co = cos_sb[:, tt, :, :].broadcast_to((128, 5, HALF))
                    si = sin_sb[:, tt, :, :].broadcast_to((128, 5, HALF))
                    rot = rot_pool.tile([128, 5, 64], f16, tag="rot")
                    t1 = tmp_pool.tile([128, 5, HALF], f32, tag="t1")
                    t2 = tmp_pool.tile([128, 5, HALF], f32, tag="t2")
                    nc.vector.tensor_tensor(t1[:], a, co, mult)
                    nc.vector.tensor_tensor(t2[:], b, si, mult)
                    nc.gpsimd.tensor_tensor(rot[:, :, 0:HALF], t1[:], t2[:], subtract)
                    t3 = tmp_pool.tile([128, 5, HALF], f32, tag="t1")
                    t4 = tmp_pool.tile([128, 5, HALF], f32, tag="t2")
                    nc.vector.tensor_tensor(t3[:], a, si, mult)
                    nc.vector.tensor_tensor(t4[:], b, co, mult)
                    nc.gpsimd.tensor_tensor(rot[:, :, HALF:D_K], t3[:], t4[:], add)
                    nc.vector.tensor_copy(v_sb[:, tt, 0:64], pp[:, 5, :])
                    prev = (rot, tt)
                flush_prev()

            # ---------------- phase 2: attention + WO partial projection
            # Pipelined at half-tile (head-pair) granularity: step s=(i,hp).
            # scores/exp/mask for step s are issued 2 steps before the
            # AV..WO tail consumes them, so every cross-engine dependency
            # has ~2 half-steps of slack and PSUM slots recycle one full
            # iteration apart.
            with ExitStack() as c2:
                sc_pool = c2.enter_context(
                    tc.tile_pool(name="scp", bufs=2, space="PSUM")
                )
                po_pool = c2.enter_context(
                    tc.tile_pool(name="pop", bufs=1, space="PSUM")
                )
                pr_pool = c2.enter_context(tc.tile_pool(name="prp", bufs=4))
                pre_pool = c2.enter_context(tc.tile_pool(name="prep", bufs=8))
                attn_pool = c2.enter_context(tc.tile_pool(name="attnp", bufs=4))
                at_pool = c2.enter_context(tc.tile_pool(name="atp", bufs=4))
                rc_pool = c2.enter_context(tc.tile_pool(name="rcp", bufs=4))
                ob_pool = c2.enter_context(tc.tile_pool(name="obp", bufs=2))

                po_tiles = {}

                def issue_scores_hp(i, hp):
                    kb0 = max(0, i - 4)
                    nkb = i - kb0 + 1
                    edge_old = i >= 4
                    # slots 0..4: score blocks; slot 5 cols 0:130 hosts the
                    # AV accumulator of the tail processed 2 steps later.
                    sc = sc_pool.tile([128, 6, 256], f32, tag="sc", name="sc")
                    for j in range(nkb):
                        kb = kb0 + j
                        nc.tensor.matmul(
                            sc[:, j, :],
                            lhsT=qk_sb[:, 4, kb * 128 : (kb + 1) * 128],
                            rhs=qk_sb[
                                :, hp * 2 : hp * 2 + 2, i * 128 : (i + 1) * 128
                            ],
                            start=True,
                            stop=True,
                        )
                    pr = pr_pool.tile([128, 5, 256], f16, tag="pr")
                    nc.scalar.activation(
                        pr[:, 0:nkb, :], sc[:, 0:nkb, :], Exp, scale=0.125
                    )
                    ed = pre_pool.tile([128, 256], f16, tag="ed")
                    nc.gpsimd.tensor_tensor(ed[:], pr[:, nkb - 1, :], md_sb[:], mult)
                    eo = None
                    if edge_old:
                        eo = pre_pool.tile([128, 256], f16, tag="eo")
                        nc.gpsimd.tensor_tensor(eo[:], pr[:, 0, :], mo_sb[:], mult)
                    return [i, hp, kb0, nkb, edge_old, pr, ed, eo, None, sc]

                def issue_tail_av(st, sc_cur):
                    i, hp, kb0, nkb, edge_old, pr, ed, eo = st[:8]
                    av = sc_cur[:, 5, 0:130].rearrange(
                        "p (two f) -> p two f", two=2
                    )
                    unmasked = [
                        j for j in range(nkb - 1) if not (j == 0 and edge_old)
                    ]
                    masked = ([0] if edge_old else []) + [nkb - 1]
                    order = unmasked + masked
                    for hq in range(2):
                        for pos, j in enumerate(order):
                            kb = kb0 + j
                            if j == nkb - 1:
                                lhs = ed[:, hq * 128 : (hq + 1) * 128]
                            elif j == 0 and edge_old:
                                lhs = eo[:, hq * 128 : (hq + 1) * 128]
                            else:
                                lhs = pr[:, j, hq * 128 : (hq + 1) * 128]
                            nc.tensor.matmul(
                                av[:, hq, :],
                                lhsT=lhs,
                                rhs=v_sb[:, kb, :],
                                start=(pos == 0),
                                stop=(pos == len(order) - 1),
                            )
                    rc = rc_pool.tile([128, 2, 1], f32, tag="rc")
                    nc.vector.reciprocal(rc[:], av[:, :, 64:65])
                    attn = attn_pool.tile([128, 2, 64], f16, tag="attn")
                    nc.vector.tensor_tensor(
                        attn[:],
                        av[:, :, 0:64],
                        rc[:, :, 0:1].broadcast_to((128, 2, 64)),
                        mult,
                    )
                    at = at_pool.tile([128, 128], f16, tag="at")
                    nc.sync.dma_start_transpose(at[:], attn[:])
                    st[8] = at

                def issue_tail_wo(st):
                    i, hp, kb0, nkb, edge_old, pr, ed, eo, at = st[:9]
                    if hp == 0:
                        po_tiles[i] = po_pool.tile(
                            [128, 2, 512], f32, tag="po", name="po"
                        )
                    po = po_tiles[i]
                    for nb in range(2):
                        nc.tensor.matmul(
                            po[:, nb, :],
                            lhsT=at[:],
                            rhs=wo_sb[:, hp, nb * 512 : (nb + 1) * 512],
                            start=(hp == 0),
                            stop=(hp == 1),
                        )
                    if hp == 1:
                        ob = ob_pool.tile([128, 1024], f16, tag="ob")
                        nc.vector.tensor_copy(ob[:], po[:])
                        nc.sync.dma_start(out_d[i * 128 : (i + 1) * 128, :], ob[:])
                        del po_tiles[i]

                pend = []
                wo_q = []
                for i in range(NT):
                    for hp in range(2):
                        st = issue_scores_hp(i, hp)
                        if len(pend) >= 2:
                            st2 = pend.pop(0)
                            issue_tail_av(st2, st[9])
                            wo_q.append(st2)
                        if len(wo_q) >= 2:
                            issue_tail_wo(wo_q.pop(0))
                        pend.append(st)
                # flush: allocate fresh sc tiles just for the AV slot
                while pend:
                    st2 = pend.pop(0)
                    scf = sc_pool.tile([128, 6, 256], f32, tag="sc", name="sc")
                    issue_tail_av(st2, scf)
                    wo_q.append(st2)
                while wo_q:
                    issue_tail_wo(wo_q.pop(0))

    nc.compile()
    return nc


def _host_inputs(x, WQ, WK, WV, WO, token_positions):
    perm64 = np.concatenate([np.arange(0, 64, 2), np.arange(1, 64, 2)])
    pos = np.asarray(token_positions).astype(np.float64)
    inv_freq = THETA ** (-np.arange(HALF, dtype=np.float64) / HALF)
    ang = pos[:, None] * inv_freq[None, :]
    cosr = np.cos(ang).astype(np.float32)
    sinr = np.sin(ang).astype(np.float32)

    def _rope_tiles(r):
        # [T, HALF] -> [128, NT, 1, HALF]
        c = r.reshape(NT, 128, HALF).transpose(1, 0, 2)
        return np.ascontiguousarray(c[:, :, None, :])

    cosb = _rope_tiles(cosr)
    sinb = _rope_tiles(sinr)

    rk = np.arange(128)[:, None]
    r = np.arange(128)[None, :]
    maskd = np.tile((rk <= r).astype(np.float32), (1, 2)).astype(np.float16)
    masko = np.tile((rk >= r).astype(np.float32), (1, 2)).astype(np.float16)
    ident16 = np.eye(128).astype(np.float16)

    in_maps = []
    for core in range(8):
        bi, g = core // 4, core % 4
        WQp = (
            WQ[g * 256 : (g + 1) * 256]
            .reshape(GROUP_SIZE, D_K, D_MODEL)[:, perm64, :]
            .reshape(256, D_MODEL)
        )
        WKp = WK[g * 64 : (g + 1) * 64][perm64, :]
        Wf = np.concatenate([WQp, WKp, WV[g * 64 : (g + 1) * 64]], axis=0)
        wqkvT = np.ascontiguousarray(Wf.T.reshape(8, 128, 384).transpose(1, 0, 2)).astype(
            np.float16
        )
        woT = np.ascontiguousarray(
            WO[:, g * 256 : (g + 1) * 256].T.reshape(2, 128, 1024).transpose(1, 0, 2)
        ).astype(np.float16)
        xT = np.ascontiguousarray(x[bi].T)
        # [1024, T] -> [NT, 128, 8, 128] -> paired tiles [NT/2, 128, 2, 8, 128]
        xt4 = (
            xT.reshape(8, 128, NT, 128)
            .transpose(2, 1, 0, 3)
            .reshape(NT // 2, 2, 128, 8, 128)
            .transpose(0, 2, 1, 3, 4)
        )
        xt4 = np.ascontiguousarray(xt4).astype(np.float16)
        in_maps.append(
            {
                "xt": xt4,
                "wqkvT": wqkvT,
                "woT": woT,
                "cosb": cosb,
                "sinb": sinb,
                "maskd": maskd,
                "masko": masko,
                "ident16": ident16,
            }
        )
    return in_maps


def kernel(x, WQ, WK, WV, WO, token_positions):
    global _PROGRAM
    from concourse.bass_utils import run_bass_kernel_spmd

    x = np.asarray(x, dtype=np.float32)
    WQ = np.asarray(WQ, dtype=np.float32)
    WK = np.asarray(WK, dtype=np.float32)
    WV = np.asarray(WV, dtype=np.float32)
    WO = np.asarray(WO, dtype=np.float32)

    if _PROGRAM is None:
        _PROGRAM = _build_program()
    nc = _PROGRAM

    in_maps = _host_inputs(x, WQ, WK, WV, WO, token_positions)
    res = run_bass_kernel_spmd(nc, in_maps, core_ids=list(range(8)))
    out = np.zeros((B, T, D_MODEL), dtype=np.float32)
    for core in range(8):
        out[core // 4] += res.results[core]["out"].astype(np.float32)
    return out


# revision 13
# speedup vs baseline: 1.0416x; 1.0416x over previous
"""GQA with RoPE + sliding-window causal attention on 8 TRN2 NeuronCores.

Sharding: batch (2) x KV-groups (4) -> 8 cores, pure SPMD (no collectives).
Each core computes q/k/v projections for its (batch, group), RoPE, windowed
attention (window=512), and a partial output projection against its group's
WO columns. Host sums the 4 group partials per batch element.

v2 layout/perf notes:
  * All 2-byte tensors are fp16 (same PE/DVE rate as bf16, 8x the mantissa):
    x tiles, W_qkv, W_o, qk_sb, v_sb, masks, probs, attn.
  * Weights pre-permuted on host so each head's 64 dims are deinterleaved
    ([even | odd]) -> RoPE is two contiguous 32-wide halves per DVE op.
  * cos/sin stored deduplicated [128, NT, 1, 32] f32 and broadcast across
    the 5 heads with a stride-0 AP dim.
  * Both phases are software-pipelined so the PE stream stays dense:
    phase 1 issues proj(i) then transpose/drain(i-1); phase 2 issues
    scores/exp/mask(i) then AV..WO(i-1).
  * WO partials drain PSUM->SBUF as fp16 into one [128,1024] buffer
    (Pool engine), one output DMA per row tile; host sums fp16 partials.
"""

import sys

sys.path.insert(0, "/opt/trn_rl_repo")

import numpy as np
from contextlib import ExitStack

D_MODEL = 1024
GROUP_SIZE = 4
NUM_GROUPS = 4
D_K = 64
THETA = 10000.0
WINDOW = 512
T = 2048
B = 2
NT = T // 128  # 16 row tiles
HALF = D_K // 2

_PROGRAM = None


def _build_program():
    from concourse import bacc, tile
    import concourse.mybir as mybir

    f32 = mybir.dt.float32
    f16 = mybir.dt.float16
    Exp = mybir.ActivationFunctionType.Exp
    mult = mybir.AluOpType.mult
    subtract = mybir.AluOpType.subtract
    add = mybir.AluOpType.add

    nc = bacc.Bacc("TRN2", target_bir_lowering=False, debug=False, num_devices=8)

    xt_d = nc.dram_tensor("xt", [NT // 2, 128, 2, 8, 128], f16, kind="ExternalInput").ap()
    wq_d = nc.dram_tensor("wqkvT", [128, 8, 384], f16, kind="ExternalInput").ap()
    wo_d = nc.dram_tensor("woT", [128, 2, 1024], f16, kind="ExternalInput").ap()
    cos_d = nc.dram_tensor("cosb", [128, NT, 1, HALF], f32, kind="ExternalInput").ap()
    sin_d = nc.dram_tensor("sinb", [128, NT, 1, HALF], f32, kind="ExternalInput").ap()
    md_d = nc.dram_tensor("maskd", [128, 256], f16, kind="ExternalInput").ap()
    mo_d = nc.dram_tensor("masko", [128, 256], f16, kind="ExternalInput").ap()
    id16_d = nc.dram_tensor("ident16", [128, 128], f16, kind="ExternalInput").ap()
    out_d = nc.dram_tensor("out", [T, D_MODEL], f16, kind="ExternalOutput").ap()

    with tile.TileContext(nc) as tc:
        with ExitStack() as ctx:
            persist = ctx.enter_context(tc.tile_pool(name="persist", bufs=1))
            wq_sb = persist.tile([128, 8, 384], f16, tag="wq")
            wo_sb = persist.tile([128, 2, 1024], f16, tag="wo")
            cos_sb = persist.tile([128, NT, 1, HALF], f32, tag="cos")
            sin_sb = persist.tile([128, NT, 1, HALF], f32, tag="sin")
            md_sb = persist.tile([128, 256], f16, tag="md")
            mo_sb = persist.tile([128, 256], f16, tag="mo")
            id16_sb = persist.tile([128, 128], f16, tag="id16")
            qk_sb = persist.tile([64, 5, T], f16, tag="qk")  # dims-major q(4)+k
            v_sb = persist.tile([128, NT, 65], f16, tag="v")  # [v | 1] per key block

            # preloads needed by phase 1 first; wo/masks late (phase 2 use)
            nc.sync.dma_start(wq_sb[:], wq_d[:])
            nc.sync.dma_start(id16_sb[:], id16_d[:])
            nc.sync.dma_start(cos_sb[:], cos_d[:])
            nc.sync.dma_start(sin_sb[:], sin_d[:])

            # ---------------- phase 1: QKV projection + RoPE + transposes
            with ExitStack() as c1:
                xt_pool = c1.enter_context(tc.tile_pool(name="xtp", bufs=2))
                rot_pool = c1.enter_context(tc.tile_pool(name="rotp", bufs=2))
                tmp_pool = c1.enter_context(tc.tile_pool(name="tmpp", bufs=2))
                pp_pool = c1.enter_context(
                    tc.tile_pool(name="ppp", bufs=2, space="PSUM")
                )
                ptr_pool = c1.enter_context(
                    tc.tile_pool(name="ptrp", bufs=2, space="PSUM")
                )

                nc.vector.memset(v_sb[:, :, 64:65], 1.0)

                prev = None  # (rot, tt) awaiting PE transpose + drain

                def flush_prev():
                    nonlocal prev
                    if prev is None:
                        return
                    rot_p, tp = prev
                    pt = ptr_pool.tile([64, 5, 128], f16, tag="pt")
                    for hh in range(5):
                        nc.tensor.transpose(pt[:, hh, :], rot_p[:, hh, :], id16_sb[:])
                    nc.scalar.copy(qk_sb[:, :, tp * 128 : (tp + 1) * 128], pt[:])
                    prev = None

                xt = None
                for tt in range(NT):
                    if tt % 2 == 0:
                        xt = xt_pool.tile([128, 2, 8, 128], f16, tag="xt")
                        nc.sync.dma_start(xt[:], xt_d[tt // 2])
                        if tt == 0:
                            nc.sync.dma_start(wo_sb[:], wo_d[:])
                            nc.sync.dma_start(md_sb[:], md_d[:])
                            nc.sync.dma_start(mo_sb[:], mo_d[:])
                    pp = pp_pool.tile([128, 6, 64], f32, tag="pp")
                    for kt in range(8):
                        nc.tensor.matmul(
                            pp[:],
                            lhsT=xt[:, tt % 2, kt, :],
                            rhs=wq_sb[:, kt, :],
                            start=(kt == 0),
                            stop=(kt == 7),
                        )
                    # PE: transposes of previous tile (its RoPE is done)
                    flush_prev()
                    # DVE/Pool: RoPE of this tile
                    a = pp[:, 0:5, 0:HALF]
                    b = pp[:, 0:5, HALF:D_K]
                    co = cos_sb[:, tt, :, :].broadcast_to((128, 5, HALF))
                    si = sin_sb[:, tt, :, :].broadcast_to((128, 5, HALF))
                    rot = rot_pool.tile([128, 5, 64], f16, tag="rot")
                    t1 = tmp_pool.tile([128, 5, HALF], f32, tag="t1")
                    t2 = tmp_pool.tile([128, 5, HALF], f32, tag="t2")
                    nc.vector.tensor_tensor(t1[:], a, co, mult)
                    nc.vector.tensor_tensor(t2[:], b, si, mult)
                    nc.gpsimd.tensor_tensor(rot[:, :, 0:HALF], t1[:], t2[:], subtract)
                    t3 = tmp_pool.tile([128, 5, HALF], f32, tag="t1")
                    t4 = tmp_pool.tile([128, 5, HALF], f32, tag="t2")
                    nc.vector.tensor_tensor(t3[:], a, si, mult)
                    nc.vector.tensor_tensor(t4[:], b, co, mult)
                    nc.gpsimd.tensor_tensor(rot[:, :, HALF:D_K], t3[:], t4[:], add)
                    nc.vector.tensor_copy(v_sb[:, tt, 0:64], pp[:, 5, :])
                    prev = (rot, tt)
                flush_prev()

            # ---------------- phase 2: attention + WO partial projection
            # Pipelined at half-tile (head-pair) granularity: step s=(i,hp).
            # scores/exp/mask for step s are issued 2 steps before the
            # AV..WO tail consumes them, so every cross-engine dependency
            # has ~2 half-steps of slack and PSUM slots recycle one full
            # iteration apart.
            with ExitStack() as c2:
                sc_pool = c2.enter_context(
                    tc.tile_pool(name="scp", bufs=2, space="PSUM")
                )
                po_pool = c2.enter_context(
                    tc.tile_pool(name="pop", bufs=2, space="PSUM")
                )
                pr_pool = c2.enter_context(tc.tile_pool(name="prp", bufs=4))
                pre_pool = c2.enter_context(tc.tile_pool(name="prep", bufs=8))
                attn_pool = c2.enter_context(tc.tile_pool(name="attnp", bufs=4))
                at_pool = c2.enter_context(tc.tile_pool(name="atp", bufs=4))
                rc_pool = c2.enter_context(tc.tile_pool(name="rcp", bufs=4))
                ob_pool = c2.enter_context(tc.tile_pool(name="obp", bufs=2))

                po_tiles = {}

                def issue_scores_hp(i, hp):
                    kb0 = max(0, i - 4)
                    nkb = i - kb0 + 1
                    edge_old = i >= 4
                    # slots 0..4: score blocks; slot 5 cols 0:130 hosts the
                    # AV accumulator of the tail processed 2 steps later.
                    sc = sc_pool.tile([128, 6, 256], f32, tag="sc", name="sc")
                    for j in range(nkb):
                        kb = kb0 + j
                        nc.tensor.matmul(
                            sc[:, j, :],
                            lhsT=qk_sb[:, 4, kb * 128 : (kb + 1) * 128],
                            rhs=qk_sb[
                                :, hp * 2 : hp * 2 + 2, i * 128 : (i + 1) * 128
                            ],
                            start=True,
                            stop=True,
                        )
                    pr = pr_pool.tile([128, 5, 256], f16, tag="pr")
                    nc.scalar.activation(
                        pr[:, 0:nkb, :], sc[:, 0:nkb, :], Exp, scale=0.125
                    )
                    ed = pre_pool.tile([128, 256], f16, tag="ed")
                    nc.gpsimd.tensor_tensor(ed[:], pr[:, nkb - 1, :], md_sb[:], mult)
                    eo = None
                    if edge_old:
                        eo = pre_pool.tile([128, 256], f16, tag="eo")
                        nc.gpsimd.tensor_tensor(eo[:], pr[:, 0, :], mo_sb[:], mult)
                    return [i, hp, kb0, nkb, edge_old, pr, ed, eo, None, sc]

                def issue_tail_av(st, sc_cur):
                    i, hp, kb0, nkb, edge_old, pr, ed, eo = st[:8]
                    av = sc_cur[:, 5, 0:130].rearrange(
                        "p (two f) -> p two f", two=2
                    )
                    unmasked = [
                        j for j in range(nkb - 1) if not (j == 0 and edge_old)
                    ]
                    masked = ([0] if edge_old else []) + [nkb - 1]
                    order = unmasked + masked
                    for hq in range(2):
                        for pos, j in enumerate(order):
                            kb = kb0 + j
                            if j == nkb - 1:
                                lhs = ed[:, hq * 128 : (hq + 1) * 128]
                            elif j == 0 and edge_old:
                                lhs = eo[:, hq * 128 : (hq + 1) * 128]
                            else:
                                lhs = pr[:, j, hq * 128 : (hq + 1) * 128]
                            nc.tensor.matmul(
                                av[:, hq, :],
                                lhsT=lhs,
                                rhs=v_sb[:, kb, :],
                                start=(pos == 0),
                                stop=(pos == len(order) - 1),
                            )
                    rc = rc_pool.tile([128, 2, 1], f32, tag="rc")
                    nc.vector.reciprocal(rc[:], av[:, :, 64:65])
                    attn = attn_pool.tile([128, 2, 64], f16, tag="attn")
                    nc.vector.tensor_tensor(
                        attn[:],
                        av[:, :, 0:64],
                        rc[:, :, 0:1].broadcast_to((128, 2, 64)),
                        mult,
                    )
                    at = at_pool.tile([128, 128], f16, tag="at")
                    nc.sync.dma_start_transpose(at[:], attn[:])
                    st[8] = at

                def issue_tail_wo(st):
                    i, hp, kb0, nkb, edge_old, pr, ed, eo, at = st[:9]
                    if hp == 0:
                        po_tiles[i] = [
                            po_pool.tile([128, 512], f32, tag="po", name="po")
                            for _ in range(2)
                        ]
                    po = po_tiles[i]
                    for nb in range(2):
                        nc.tensor.matmul(
                            po[nb][:],
                            lhsT=at[:],
                            rhs=wo_sb[:, hp, nb * 512 : (nb + 1) * 512],
                            start=(hp == 0),
                            stop=(hp == 1),
                        )
                    if hp == 1:
                        ob = ob_pool.tile([128, 1024], f16, tag="ob")
                        for nb in range(2):
                            nc.vector.tensor_copy(
                                ob[:, nb * 512 : (nb + 1) * 512], po[nb][:]
                            )
                        nc.sync.dma_start(out_d[i * 128 : (i + 1) * 128, :], ob[:])
                        del po_tiles[i]

                pend = []
                wo_q = []
                for i in range(NT):
                    for hp in range(2):
                        st = issue_scores_hp(i, hp)
                        if len(pend) >= 2:
                            st2 = pend.pop(0)
                            issue_tail_av(st2, st[9])
                            wo_q.append(st2)
                        if len(wo_q) >= 3:
                            issue_tail_wo(wo_q.pop(0))
                        pend.append(st)
                # flush: allocate fresh sc tiles just for the AV slot
                while pend:
                    st2 = pend.pop(0)
                    scf = sc_pool.tile([128, 6, 256], f32, tag="sc", name="sc")
                    issue_tail_av(st2, scf)
                    wo_q.append(st2)
                while wo_q:
                    issue_tail_wo(wo_q.pop(0))

    nc.compile()
    return nc


def _host_inputs(x, WQ, WK, WV, WO, token_positions):
    perm64 = np.concatenate([np.arange(0, 64, 2), np.arange(1, 64, 2)])
    pos = np.asarray(token_positions).astype(np.float64)
    inv_freq = THETA ** (-np.arange(HALF, dtype=np.float64) / HALF)
    ang = pos[:, None] * inv_freq[None, :]
    cosr = np.cos(ang).astype(np.float32)
    sinr = np.sin(ang).astype(np.float32)

    def _rope_tiles(r):
        # [T, HALF] -> [128, NT, 1, HALF]
        c = r.reshape(NT, 128, HALF).transpose(1, 0, 2)
        return np.ascontiguousarray(c[:, :, None, :])

    cosb = _rope_tiles(cosr)
    sinb = _rope_tiles(sinr)

    rk = np.arange(128)[:, None]
    r = np.arange(128)[None, :]
    maskd = np.tile((rk <= r).astype(np.float32), (1, 2)).astype(np.float16)
    masko = np.tile((rk >= r).astype(np.float32), (1, 2)).astype(np.float16)
    ident16 = np.eye(128).astype(np.float16)

    in_maps = []
    for core in range(8):
        bi, g = core // 4, core % 4
        WQp = (
            WQ[g * 256 : (g + 1) * 256]
            .reshape(GROUP_SIZE, D_K, D_MODEL)[:, perm64, :]
            .reshape(256, D_MODEL)
        )
        WKp = WK[g * 64 : (g + 1) * 64][perm64, :]
        Wf = np.concatenate([WQp, WKp, WV[g * 64 : (g + 1) * 64]], axis=0)
        wqkvT = np.ascontiguousarray(Wf.T.reshape(8, 128, 384).transpose(1, 0, 2)).astype(
            np.float16
        )
        woT = np.ascontiguousarray(
            WO[:, g * 256 : (g + 1) * 256].T.reshape(2, 128, 1024).transpose(1, 0, 2)
        ).astype(np.float16)
        xT = np.ascontiguousarray(x[bi].T)
        # [1024, T] -> [NT, 128, 8, 128] -> paired tiles [NT/2, 128, 2, 8, 128]
        xt4 = (
            xT.reshape(8, 128, NT, 128)
            .transpose(2, 1, 0, 3)
            .reshape(NT // 2, 2, 128, 8, 128)
            .transpose(0, 2, 1, 3, 4)
        )
        xt4 = np.ascontiguousarray(xt4).astype(np.float16)
        in_maps.append(
            {
                "xt": xt4,
                "wqkvT": wqkvT,
                "woT": woT,
                "cosb": cosb,
                "sinb": sinb,
                "maskd": maskd,
                "masko": masko,
                "ident16": ident16,
            }
        )
    return in_maps


def kernel(x, WQ, WK, WV, WO, token_positions):
    global _PROGRAM
    from concourse.bass_utils import run_bass_kernel_spmd

    x = np.asarray(x, dtype=np.float32)
    WQ = np.asarray(WQ, dtype=np.float32)
    WK = np.asarray(WK, dtype=np.float32)
    WV = np.asarray(WV, dtype=np.float32)
    WO = np.asarray(WO, dtype=np.float32)

    if _PROGRAM is None:
        _PROGRAM = _build_program()
    nc = _PROGRAM

    in_maps = _host_inputs(x, WQ, WK, WV, WO, token_positions)
    res = run_bass_kernel_spmd(nc, in_maps, core_ids=list(range(8)))
    out = np.zeros((B, T, D_MODEL), dtype=np.float32)
    for core in range(8):
        out[core // 4] += res.results[core]["out"].astype(np.float32)
    return out
